# revision 2
# baseline (speedup 1.0000x reference)
"""Trainium2 Bass kernel for nn_HadamardModule (SORF random-feature module), v2.

Reference computation:
    x_ = x @ projector                      # [N=8192, 128]
    for t in 0,1: y = COEFF * fwht(d[t] * y)   (64 stacks share x_)
    out = cos(feats + 2*pi*b)

Math: fwht(128) == multiply by symmetric Hadamard H. Per stack s:
    feats_s = C^2 * D0_s H D1_s H x_   (as columns)
Restructured so the per-stack fold needs NO matmul and NO PSUM evacuation:
    y1   = H @ x_^T               # shared across stacks (one matmul)
    zraw = (d1_s*H)^T ... i.e. matmul with stationary w1_s = d1_s (*) H
           (row-scale of H: one GPSIMD tensor_scalar per stack)
    phase (periods) y = (C^2/2pi)*d0_s[k] * zraw[k,row] + c_s[k]
    r = y - round(y)              # exact fp32 magic-number trick
    out = sin(2*pi*r)             # == cos(feats + 2pi*b), c = b + 0.25 (frac)
The scale/bias/round/subtract run as ONE custom DVE op (registered here):
    body: y = Src0*C0 + C1; t = (y + C2) - C2; out = y - t
reading zraw straight out of PSUM and writing r in place; the ScalarE Sin
LUT then produces bf16 output. Per output element: 1 DVE pass + 1 ACT pass
(the old pipeline used 4 passes plus a 3-op/stack fold).

Sharding: data-parallel over the 8192 rows -> 1024 rows per core on 8 cores.
"""

import concurrent.futures as _futures

import numpy as np

NPCAS = 128
OUT_DIM = 8192
NSTACKS = 64
COEFF = np.sqrt(np.float64(NPCAS)) / 3.0
TWO_PI = 2.0 * np.pi
C_SCALE = float(COEFF**2 / TWO_PI)
N_CORES = 8
ROWS = 8192
ROWS_PER_CORE = ROWS // N_CORES  # 1024
CHUNK = 512
N_CHUNKS = ROWS_PER_CORE // CHUNK  # 2
MAGIC = float(np.float32(1.5 * 2**23))

_cached = {}


def _hadamard128():
    H = np.array([[1.0]])
    while H.shape[0] < NPCAS:
        H = np.block([[H, H], [H, -H]])
    return H


def _get_frac_op():
    """Register (once) the fused scale+bias+round-subtract custom DVE op.

    out = y - ((y + M) - M),  y = in0*s0 + s1   -> y - round(y) in [-0.5,0.5]
    (M = 1.5*2^23; each DVE ALU stage rounds to fp32, which is what makes
    the magic-number round exact for |y| < 2^22.)
    """
    if "frac_op" in _cached:
        return _cached["frac_op"]
    import concourse.dve_ops as dve_ops
    from concourse.dve_uop import DveOpSpec

    name = "FRAC_AFFINE_ANT"
    existing = [op for op in dve_ops.OPS if op.name == name]
    if existing:
        _cached["frac_op"] = existing[0]
        return existing[0]

    Src0, C0, C1, C2 = dve_ops.Src0, dve_ops.C0, dve_ops.C1, dve_ops.C2
    _y = Src0 * C0 + C1
    body = _y - ((_y + C2) - C2)

    def _ref(in0, in1, s0, s1, imm2):
        y = (in0.astype(np.float32) * np.float32(s0)).astype(np.float32)
        y = (y + np.float32(s1)).astype(np.float32)
        t = (y + np.float32(imm2)).astype(np.float32)
        t = (t - np.float32(imm2)).astype(np.float32)
        return (y - t).astype(np.float32)

    spec = dve_ops.Spec(body=body, reference=_ref)
    opcode = dve_ops._CUSTOM_DVE_ROW_BASE + len(dve_ops.OPS)
    assert opcode < 0x20
    dve_ops._SUB_OPCODE_FOR_NAME[name] = opcode

    from concourse.dve_table_gen import dve_ver_for

    ver = dve_ver_for("TRN2")
    uops = dve_ops.lower(spec, ver=ver)
    sha = DveOpSpec(
        name=name, opcode=opcode, uops=uops, rd1_en=dve_ops.has_src1(spec)
    ).sha(ver)
    op = dve_ops.DveOp(name, spec, subdim=False, uops_sha={ver: sha})
    dve_ops.OPS.append(op)
    dve_ops.CUSTOM_DVE_SPECS[name] = spec
    _cached["frac_op"] = op
    return op


def _build_nc(reps=1, mode="full"):
    import concourse.bacc as bacc
    import concourse.mybir as mybir
    import concourse.tile as tile

    frac_op = _get_frac_op()

    f32 = mybir.dt.float32
    bf16 = mybir.dt.bfloat16
    mult = mybir.AluOpType.mult

    nc = bacc.Bacc("TRN2", target_bir_lowering=False, debug=False)
    xT = nc.dram_tensor("xT", [4, 128, ROWS_PER_CORE], f32, kind="ExternalInput")
    Pc = nc.dram_tensor("Pc", [4, 128, 128], f32, kind="ExternalInput")
    Hd = nc.dram_tensor("Hd", [128, 128], f32, kind="ExternalInput")
    d1d = nc.dram_tensor("d1d", [128, NSTACKS], f32, kind="ExternalInput")
    d0cd = nc.dram_tensor("d0cd", [128, NSTACKS], f32, kind="ExternalInput")
    cbd = nc.dram_tensor("cbd", [128, NSTACKS], f32, kind="ExternalInput")
    out = nc.dram_tensor(
        "out", [NSTACKS, 128, ROWS_PER_CORE], bf16, kind="ExternalOutput"
    )

    with tile.TileContext(nc) as tc:
        with (
            tc.tile_pool(name="const", bufs=1) as const,
            tc.tile_pool(name="psum_fp", bufs=2, space="PSUM") as psum_fp,
            tc.tile_pool(name="psum_z", bufs=6, space="PSUM") as psum_z,
            tc.tile_pool(name="outp", bufs=8) as outp,
        ):
            for _rep in range(reps):
                Pt = const.tile([128, 4, 128], f32, tag="Pt")
                Xt = const.tile([128, 4, ROWS_PER_CORE], f32, tag="Xt")
                for k in range(4):
                    nc.sync.dma_start(Pt[:, k, :], Pc[k])
                    nc.sync.dma_start(Xt[:, k, :], xT[k])
                Ht = const.tile([128, 128], f32, tag="Ht")
                nc.sync.dma_start(Ht[:], Hd[:])
                d1t = const.tile([128, NSTACKS], f32, tag="d1t")
                d0ct = const.tile([128, NSTACKS], f32, tag="d0ct")
                cbt = const.tile([128, NSTACKS], f32, tag="cbt")
                nc.sync.dma_start(d1t[:], d1d[:])
                nc.sync.dma_start(d0ct[:], d0cd[:])
                nc.sync.dma_start(cbt[:], cbd[:])

                # projection: x_^T = P^T @ x^T  -> xsb [128=pca, 2, 512]
                xsb = const.tile([128, N_CHUNKS, CHUNK], f32, tag="xsb")
                for c in range(N_CHUNKS):
                    pp = psum_fp.tile([128, CHUNK], f32, tag="fp")
                    for k in range(4):
                        nc.tensor.matmul(
                            pp[:],
                            Pt[:, k, :],
                            Xt[:, k, c * CHUNK : (c + 1) * CHUNK],
                            start=(k == 0),
                            stop=(k == 3),
                        )
                    nc.scalar.copy(xsb[:, c, :], pp[:])

                # per-stack w1_s = d1_s (*) H (row-scale), on GPSIMD
                w1t = const.tile([128, NSTACKS, 128], f32, tag="w1t")
                for s in range(NSTACKS):
                    nc.gpsimd.tensor_scalar(
                        w1t[:, s, :], Ht[:], d1t[:, s : s + 1], None, mult
                    )

                # fold: G_s = H @ w1_s (4 stacks per PSUM tile), then the
                # ACT evacuation applies the per-partition c*d0_s row-scale:
                # At_s = d0c_s (*) G_s  (G symmetric -> this puts D0 on the
                # contraction side of the main matmul, matching H D1 H D0 x_)
                At = const.tile([128, NSTACKS, 128], f32, tag="At")
                for g in range(NSTACKS // 4):
                    pin = psum_fp.tile([128, CHUNK], f32, tag="fp")
                    nc.tensor.matmul(
                        pin[:],
                        Ht[:],
                        w1t[:, 4 * g : 4 * g + 4, :],
                        start=True,
                        stop=True,
                    )
                    for j in range(4):
                        s = 4 * g + j
                        nc.scalar.mul(
                            At[:, s, :],
                            pin[:, 128 * j : 128 * (j + 1)],
                            d0ct[:, s : s + 1],
                        )

                # main loop at half-stack [128, 512] granularity (1 PSUM
                # bank per tile, 6-deep ring): matmul -> fused frac
                # (in-place in PSUM) -> Sin -> DMA out
                for s in range(NSTACKS):
                    for c in range(N_CHUNKS):
                        z = psum_z.tile([128, CHUNK], f32)
                        nc.tensor.matmul(
                            z[:],
                            At[:, s, :],
                            xsb[:, c, :],
                            start=True,
                            stop=True,
                        )
                        if mode == "mm" and not (
                            s == NSTACKS - 1 and c == N_CHUNKS - 1
                        ):
                            continue
                        nc.vector._custom_dve(
                            frac_op,
                            out=z[:],
                            in0=z[:],
                            s0=1.0,
                            s1=cbt[:, s : s + 1],
                            imm2=MAGIC,
                        )
                        if mode == "nosin" and not (
                            s == NSTACKS - 1 and c == N_CHUNKS - 1
                        ):
                            continue
                        osb = outp.tile([128, CHUNK], bf16)
                        nc.scalar.activation(
                            osb[:],
                            z[:],
                            mybir.ActivationFunctionType.Sin,
                            bias=0.0,
                            scale=TWO_PI,
                        )
                        nc.sync.dma_start(
                            out[s, :, c * CHUNK : (c + 1) * CHUNK], osb[:]
                        )

    nc.compile()
    return nc


def _make_runner(reps=1, mode="full"):
    """Compile once and build a persistent jitted SPMD executable."""
    import jax
    import concourse.mybir as mybir
    from jax.experimental.shard_map import shard_map
    from jax.sharding import Mesh, NamedSharding, PartitionSpec
    from concourse.bass2jax import (
        _bass_exec_p,
        install_neuronx_cc_hook,
        partition_id_tensor,
    )

    nc = _build_nc(reps=reps, mode=mode)
    install_neuronx_cc_hook()

    partition_name = (
        nc.partition_id_tensor.name if nc.partition_id_tensor else None
    )
    in_names, out_names, out_avals = [], [], []
    for alloc in nc.m.functions[0].allocations:
        if not isinstance(alloc, mybir.MemoryLocationSet):
            continue
        name = alloc.memorylocations[0].name
        if alloc.kind == "ExternalInput":
            if name != partition_name:
                in_names.append(name)
        elif alloc.kind == "ExternalOutput":
            out_names.append(name)
            out_avals.append(
                jax.core.ShapedArray(
                    tuple(alloc.tensor_shape), mybir.dt.np(alloc.dtype)
                )
            )

    sharded_inputs = {"xT"}
    call_names = tuple(in_names) + tuple(out_names)
    if partition_name is not None:
        call_names = call_names + (partition_name,)

    def _body(*args):
        extra = [partition_id_tensor()] if partition_name is not None else []
        outs = _bass_exec_p.bind(
            *args,
            *extra,
            out_avals=tuple(out_avals),
            in_names=call_names,
            out_names=tuple(out_names),
            lowering_input_output_aliases=(),
            sim_require_finite=True,
            sim_require_nnan=True,
            nc=nc,
        )
        return tuple(outs)

    devices = jax.devices()[:N_CORES]
    mesh = Mesh(np.asarray(devices), ("core",))
    in_specs = tuple(
        PartitionSpec("core") if n in sharded_inputs else PartitionSpec()
        for n in in_names
    ) + (PartitionSpec("core"),) * len(out_names)
    out_specs = (PartitionSpec("core"),) * len(out_names)
    fn = jax.jit(
        shard_map(
            _body, mesh=mesh, in_specs=in_specs, out_specs=out_specs, check_rep=False
        )
    )

    zeros = [
        jax.device_put(
            np.zeros((N_CORES * a.shape[0], *a.shape[1:]), a.dtype),
            NamedSharding(mesh, PartitionSpec("core")),
        )
        for a in out_avals
    ]
    return fn, in_names, zeros


def _get_runner(reps=1, mode="full"):
    key = ("runner", reps, mode)
    if key not in _cached:
        _cached[key] = _make_runner(reps=reps, mode=mode)
    return _cached[key]


def _host_prep(x, projector, d, b):
    H = np.ascontiguousarray(_hadamard128(), dtype=np.float32)
    d64 = np.asarray(d, np.float64)
    d0c = np.ascontiguousarray((d64[0].T * C_SCALE).astype(np.float32))  # [128,64]
    d1 = np.ascontiguousarray(d64[1].T.astype(np.float32))  # [128, 64]

    # phase bias in periods: c = b + 0.25 (cos -> sin), centered to [-0.5,0.5]
    bp = np.asarray(b, np.float64) + 0.25
    bpp = bp - np.round(bp)
    cb = np.ascontiguousarray(bpp.reshape(NSTACKS, 128).T.astype(np.float32))

    Pc = np.ascontiguousarray(np.asarray(projector, np.float32).reshape(4, 128, 128))

    x2 = np.asarray(x, np.float32).reshape(ROWS, 512)
    xT = np.empty((N_CORES, 4, 128, ROWS_PER_CORE), np.float32)
    for core in range(N_CORES):
        xs = x2[core * ROWS_PER_CORE : (core + 1) * ROWS_PER_CORE]
        xT[core] = xs.T.reshape(4, 128, ROWS_PER_CORE)
    xT = xT.reshape(N_CORES * 4, 128, ROWS_PER_CORE)
    return {
        "xT": xT, "Pc": Pc, "Hd": H, "d1d": d1, "d0cd": d0c, "cbd": cb
    }


def _assemble(out_global):
    """core-sharded [8*64, 128, 1024] bf16 -> [64, 128, 8192] fp32."""
    full = np.empty((ROWS, OUT_DIM), np.float32)
    view = full.reshape(N_CORES, ROWS_PER_CORE, NSTACKS, 128)

    shards = sorted(
        out_global.addressable_shards, key=lambda s: s.index[0].start or 0
    )

    def fetch(i):
        o = np.asarray(shards[i].data)  # [64, 128, 1024] bf16
        np.copyto(view[i], o.transpose(2, 0, 1))

    with _futures.ThreadPoolExecutor(max_workers=N_CORES) as ex:
        list(ex.map(fetch, range(N_CORES)))
    return full.reshape(64, 128, OUT_DIM)


def kernel(x, projector, d, b):
    fn, in_names, zeros = _get_runner(reps=1)
    ins = _host_prep(
        np.asarray(x), np.asarray(projector), np.asarray(d), np.asarray(b)
    )
    outs = fn(*[ins[n] for n in in_names], *zeros)
    return _assemble(outs[0])
